# revision 30
# baseline (speedup 1.0000x reference)
"""Energy Transformer descent kernel for 8 Trainium2 NeuronCores (v2).

Problem: 12 steps of gradient descent on
  E(x) = -(1/beta) sum logsumexp(beta q k^T) - 0.5 sum relu(g xi^T)^2,
  g = LayerNorm(x; gamma, delta), q = g Wq_h, k = g Wk_h.

Sharding: data-parallel over batch B=4 -> core pairs (2b, 2b+1); within a
pair, core j takes attention heads j*6..j*6+5 and Hopfield memories
xi[j*1536:(j+1)*1536].  Both energy terms contribute additively to dE/dx
and LayerNorm-backward is linear in the upstream gradient, so each core
computes a partial dx and a pairwise AllReduce produces the full step.

v2 changes vs the baseline:
- P^T is never formed by PE transposes.  Instead E = exp(S) and
  ET = exp(S^T) are both computed by matmul (S^T costs one extra C=64
  matmul per chunk), kept UNnormalized; the softmax 1/Z is folded
  per-partition into q (for dK^T) and applied per-column to dQ^T via a
  row-replicated 1/Z tile built with tiny PE column-transposes + one
  gpsimd partition_broadcast.
- Hopfield h-pass is interleaved with the attention heads (PE filler
  during softmax waits); xi/xi^T live resident in SBUF.
- dgT accumulation is d-chunk-outer so each PSUM bank evacuates as soon
  as its chain stops; the tail transposes overlap remaining banks.
- Step tail split around ONE AllReduce: pass 1 (LN-bwd via fused
  ln_bwd_dx, rstd folded into the update scale) is AR-independent;
  pass 2 fuses the x update with next step's LN stats (update accum =
  sum(x), ACT Square accum = sum(x^2), DVE-Newton rsqrt - no ACT table
  switches anywhere), chunk 0 at high priority so the next step's PE
  work restarts right after the AllReduce.

Host-side preprocessing folds gamma and the attention scale into the
weights (delta must be zero, which the problem guarantees).
"""

import numpy as np

import concourse.bass as bass
import concourse.tile as tile
from concourse import bacc, mybir

STEPS = 12
ALPHA = 0.125
EPS = 1e-5
B, N, D, H, HD, M = 4, 512, 768, 12, 64, 3072
P = 128
NT = N // P  # 4 row chunks
DT = D // P  # 6 embed chunks
HL = H // 2  # heads per core
EW = HL * HD  # 384 local head width
ET = EW // P  # 3 stacked head-pair chunks
ML = M // 2  # memories per core
MT = ML // P  # 12 memory chunks
F32 = mybir.dt.float32
F32R = mybir.dt.float32r
BF16 = mybir.dt.bfloat16
AF = mybir.ActivationFunctionType
OP = mybir.AluOpType

REPLICA_GROUPS = [[0, 1], [2, 3], [4, 5], [6, 7]]


def f_(ap):
    return ap.bitcast(F32)


def build_kernel(steps=STEPS, with_ar=True):
    nc = bacc.Bacc("TRN2", target_bir_lowering=False, debug=False, num_devices=8)

    x_in = nc.declare_dram_parameter("x", [N, D], F32, isOutput=False)
    wq_d = nc.declare_dram_parameter("wq", [D, EW], BF16, isOutput=False)
    wk_d = nc.declare_dram_parameter("wk", [D, EW], BF16, isOutput=False)
    wqt_d = nc.declare_dram_parameter("wqt", [EW, D], BF16, isOutput=False)
    wkt_d = nc.declare_dram_parameter("wkt", [EW, D], BF16, isOutput=False)
    xi_d = nc.declare_dram_parameter("xi", [ML, D], BF16, isOutput=False)
    xit_d = nc.declare_dram_parameter("xit", [D, ML], BF16, isOutput=False)
    x_out = nc.declare_dram_parameter("x_out", [N, D], F32, isOutput=True)

    with tile.TileContext(nc) as tc:
        import contextlib

        with contextlib.ExitStack() as ctx:
            consts = ctx.enter_context(tc.tile_pool(name="consts", bufs=1))
            work = ctx.enter_context(tc.tile_pool(name="work", bufs=1))
            attp = ctx.enter_context(tc.tile_pool(name="attp", bufs=2))
            stats = ctx.enter_context(tc.tile_pool(name="stats", bufs=4))
            rtp = ctx.enter_context(tc.tile_pool(name="rtp", bufs=1))
            scr = ctx.enter_context(tc.tile_pool(name="scr", bufs=2))
            ps = ctx.enter_context(tc.tile_pool(name="ps", bufs=2, space="PSUM"))
            drp = ctx.enter_context(tc.tile_pool(name="drp", bufs=2, space="DRAM"))

            # ---- resident tensors ----
            wq_sb = consts.tile([P, DT, EW], BF16)
            nc.sync.dma_start(out=wq_sb[:], in_=wq_d.rearrange("(dt p) e -> p dt e", p=P))
            wk_sb = consts.tile([P, DT, EW], BF16)
            nc.sync.dma_start(out=wk_sb[:], in_=wk_d.rearrange("(dt p) e -> p dt e", p=P))
            wqt_sb = consts.tile([P, ET, D], BF16)
            nc.sync.dma_start(out=wqt_sb[:], in_=wqt_d.rearrange("(et p) d -> p et d", p=P))
            wkt_sb = consts.tile([P, ET, D], BF16)
            nc.sync.dma_start(out=wkt_sb[:], in_=wkt_d.rearrange("(et p) d -> p et d", p=P))
            x_sb = consts.tile([P, NT, D], F32)
            nc.sync.dma_start(out=x_sb[:], in_=x_in.rearrange("(nt p) d -> p nt d", p=P))
            xit_sb = consts.tile([P, DT, ML], BF16)
            nc.sync.dma_start(out=xit_sb[:], in_=xit_d.rearrange("(dt p) m -> p dt m", p=P))
            xi_sb = consts.tile([P, MT, D], BF16)
            nc.sync.dma_start(out=xi_sb[:], in_=xi_d.rearrange("(mt p) d -> p mt d", p=P))

            from concourse.masks import make_identity

            ident_f = consts.tile([P, P], F32)
            make_identity(nc, ident_f[:])
            ident = consts.tile([P, P], F32R)
            nc.vector.tensor_copy(out=ident[:], in_=ident_f[:])
            ident_b = consts.tile([P, P], BF16)
            nc.vector.tensor_copy(out=ident_b[:], in_=ident_f[:])
            eps_t = consts.tile([P, 1], F32)
            nc.vector.memset(eps_t[:], EPS)

            def ln_stats(nt, mu_t, var_t):
                """bn stats for row-chunk nt -> mean, var columns."""
                xt = x_sb[:, nt, :]
                st = stats.tile([P, 3, 6], F32, tag="bnst")
                xg = xt.rearrange("p (g s) -> p g s", s=256)
                for gs in range(3):
                    nc.vector.bn_stats(out=st[:, gs, :], in_=xg[:, gs, :])
                mv = stats.tile([P, 2], F32, tag="mv")
                nc.vector.bn_aggr(out=mv[:], in_=st[:])
                nc.vector.tensor_copy(out=mu_t[:, nt : nt + 1], in_=mv[:, 0:1])
                nc.vector.tensor_copy(out=var_t[:, nt : nt + 1], in_=mv[:, 1:2])

            def ln_rstd(nt, var_ap, rstd_t):
                """rstd = 1/sqrt(var+eps): linear seed 1.5-0.5(v+eps) + 2
                Newton steps (var stays in [0.8, 1.25] here; no ACT table)."""
                rr = rstd_t[:, nt : nt + 1]
                vh = stats.tile([P, 1], F32, tag="vh")
                nc.vector.tensor_scalar(
                    out=vh[:], in0=var_ap, scalar1=-0.5, scalar2=-0.5 * EPS,
                    op0=OP.mult, op1=OP.add,
                )
                nc.vector.tensor_scalar_add(out=rr, in0=vh[:], scalar1=1.5)
                for _ in range(2):
                    y2 = stats.tile([P, 1], F32, tag="y2")
                    nc.vector.tensor_tensor(out=y2[:], in0=rr, in1=rr, op=OP.mult)
                    nc.vector.tensor_scalar(
                        out=y2[:], in0=y2[:], scalar1=vh[:], scalar2=1.5,
                        op0=OP.mult, op1=OP.add,
                    )
                    nc.vector.tensor_tensor(out=rr, in0=rr, in1=y2[:], op=OP.mult)

            def ln_xhat(nt, mu_ap, rstd_t, xhat_t):
                rr = rstd_t[:, nt : nt + 1]
                nmu = stats.tile([P, 1], F32, tag="nmu")
                nc.vector.scalar_tensor_tensor(
                    out=nmu[:], in0=mu_ap, scalar=-1.0, in1=rr, op0=OP.mult, op1=OP.mult,
                )
                nc.scalar.activation(
                    out=xhat_t[:, nt, :], in_=x_sb[:, nt, :], func=AF.Identity, scale=rr, bias=nmu[:],
                )

            # LN-fwd for step 0
            xhat = work.tile([P, NT, D], F32R, tag="xhat")
            rstd = stats.tile([P, NT], F32, tag="rstd")
            mu0 = stats.tile([P, NT], F32, tag="mu")
            var0 = stats.tile([P, NT], F32, tag="var")
            for nt in range(NT):
                ln_stats(nt, mu0, var0)
                ln_rstd(nt, var0[:, nt : nt + 1], rstd)
                ln_xhat(nt, mu0[:, nt : nt + 1], rstd, xhat)

            for step in range(steps):
                # ======== gT = xhat^T [d-part, n-free] ========
                psw_ctx = tc.tile_pool(name="psw", bufs=6, space="PSUM")
                psw = psw_ctx.__enter__()
                gT = work.tile([P, DT, N], BF16, tag="gT")
                # nt-outer so each row-chunk's transposes unblock right after
                # that chunk's xhat lands (pipelines across the step boundary)
                for nt in range(NT):
                    pa = psw.tile([P, 512], F32R, tag="psw")
                    for dt in range(4):
                        nc.tensor.transpose(pa[:, dt * P : (dt + 1) * P], xhat[:, nt, dt * P : (dt + 1) * P], ident[:])
                    pb = psw.tile([P, 512], F32R, tag="psw")
                    for dt in range(4, DT):
                        nc.tensor.transpose(pb[:, (dt - 4) * P : (dt - 3) * P], xhat[:, nt, dt * P : (dt + 1) * P], ident[:])
                    nc.vector.tensor_copy(out=gT[:, 0:4, nt * P : (nt + 1) * P], in_=pa[:].rearrange("p (dt c) -> p dt c", c=P))
                    nc.vector.tensor_copy(out=gT[:, 4:6, nt * P : (nt + 1) * P], in_=pb[:, 0:256].rearrange("p (dt c) -> p dt c", c=P))

                # ======== projections q, k [n-part, e] ========
                q = work.tile([P, NT, EW], BF16, tag="q")
                k = work.tile([P, NT, EW], BF16, tag="k")
                for nt in range(NT):
                    ppq = psw.tile([P, 512], F32, tag="psw")
                    ppk = psw.tile([P, 512], F32, tag="psw")
                    for dt in range(DT):
                        lh = gT[:, dt, nt * P : (nt + 1) * P]
                        nc.tensor.matmul(ppq[:, :EW], lh, wq_sb[:, dt, :], start=(dt == 0), stop=(dt == DT - 1))
                        nc.tensor.matmul(ppk[:, :EW], lh, wk_sb[:, dt, :], start=(dt == 0), stop=(dt == DT - 1))
                    nc.vector.tensor_copy(out=q[:, nt, :], in_=ppq[:, :EW])
                    nc.vector.tensor_copy(out=k[:, nt, :], in_=ppk[:, :EW])
                # qT, kT [e-part, n]
                qT = work.tile([P, ET, N], BF16, tag="qT")
                kT = work.tile([P, ET, N], BF16, tag="kT")
                for dst, srct in ((qT, q), (kT, k)):
                    for et in range(ET):
                        pp = psw.tile([P, 512], BF16, tag="psw")
                        for nt in range(NT):
                            nc.tensor.transpose(
                                pp[:, nt * P : (nt + 1) * P],
                                srct[:, nt, et * P : (et + 1) * P], ident_b[:],
                            )
                        nc.vector.tensor_copy(out=dst[:, et, :], in_=pp[:])

                # ======== attention heads (+ interleaved hopfield h-pass) ========
                RT = rtp.tile([P, MT, N], BF16, tag="RT")
                dqTst = work.tile([P, ET, N], BF16, tag="dqTst")
                dkTst = work.tile([P, ET, N], BF16, tag="dkTst")
                for h in range(HL):
                    et, eo = h // 2, (h % 2) * HD
                    qTh = qT[eo : eo + HD, et, :]
                    kTh = kT[eo : eo + HD, et, :]
                    # E = exp(S) rows, unnormalized; Z row-sums
                    Eu = attp.tile([P, NT, N], BF16, tag="Eu")
                    ETu = attp.tile([P, NT, N], BF16, tag="ETu")
                    zz = stats.tile([P, NT], F32, tag="zz")
                    for nt in range(NT):
                        sc = psw.tile([P, 512], F32, tag="psw")
                        nc.tensor.matmul(
                            sc[:], qT[eo : eo + HD, et, nt * P : (nt + 1) * P], kTh,
                            start=True, stop=True,
                        )
                        nc.scalar.activation(
                            out=Eu[:, nt, :], in_=sc[:], func=AF.Exp, bias=0.0, scale=1.0,
                            accum_out=zz[:, nt : nt + 1],
                        )
                    rz = stats.tile([P, NT], F32, tag="rz")
                    nc.vector.reciprocal(out=rz[:], in_=zz[:])
                    # row-replicated 1/Z tile [128, N]: column transposes to [1, N],
                    # then one gpsimd partition broadcast
                    rzps = psw.tile([P, 512], F32, tag="psw")
                    for c in range(NT):
                        nc.tensor.transpose(rzps[0:1, c * P : (c + 1) * P], rz[:, c : c + 1], ident_f[:])
                    rzT = stats.tile([1, N], F32, tag="rzT")
                    nc.vector.tensor_copy(out=rzT[:], in_=rzps[0:1, 0:N])
                    rzb = stats.tile([P, N], F32, tag="rzb")
                    nc.gpsimd.partition_broadcast(rzb[:, :], rzT[0:1, :])
                    # ET = exp(S^T), unnormalized
                    for mt in range(NT):
                        scT = psw.tile([P, 512], F32, tag="psw")
                        nc.tensor.matmul(
                            scT[:], kT[eo : eo + HD, et, mt * P : (mt + 1) * P], qTh,
                            start=True, stop=True,
                        )
                        nc.scalar.activation(out=ETu[:, mt, :], in_=scT[:], func=AF.Exp)
                    # dqT_h = (sum_mt k_h[mt]^T ET[mt]) * rzb
                    pp = psw.tile([P, 512], F32, tag="psw")
                    for mt in range(NT):
                        nc.tensor.matmul(
                            pp[:HD, :], k[:, mt, h * HD : (h + 1) * HD], ETu[:, mt, :],
                            start=(mt == 0), stop=(mt == NT - 1),
                        )
                    nc.vector.tensor_tensor(
                        out=dqTst[eo : eo + HD, et, :], in0=pp[:HD, :], in1=rzb[0:HD, :], op=OP.mult,
                    )
                    # dkT_h = sum_nt (q_h[nt] * rz)^T E[nt]
                    qs = scr.tile([P, NT, HD], BF16, tag="qs")
                    for nt in range(NT):
                        nc.vector.tensor_scalar_mul(
                            out=qs[:, nt, :], in0=q[:, nt, h * HD : (h + 1) * HD],
                            scalar1=rz[:, nt : nt + 1],
                        )
                    pp2 = psw.tile([P, 512], F32, tag="psw")
                    for nt in range(NT):
                        nc.tensor.matmul(
                            pp2[:HD, :], qs[:, nt, :], Eu[:, nt, :],
                            start=(nt == 0), stop=(nt == NT - 1),
                        )
                    nc.vector.tensor_copy(out=dkTst[eo : eo + HD, et, :], in_=pp2[:HD, :])
                    # hopfield h-pass chunks riding along with this head
                    for mt in (2 * h, 2 * h + 1):
                        hp = ps.tile([P, 512], F32, tag="ps")
                        for dt in range(DT):
                            nc.tensor.matmul(
                                hp[:], xit_sb[:, dt, mt * P : (mt + 1) * P], gT[:, dt, :],
                                start=(dt == 0), stop=(dt == DT - 1),
                            )
                        nc.scalar.activation(out=RT[:, mt, :], in_=hp[:], func=AF.Relu)

                psw_ctx.__exit__(None, None, None)
                # ======== dg accumulation in PSUM, transposed [d-chunk, n] ========
                psdg_ctx = tc.tile_pool(name="psdg", bufs=1, space="PSUM")
                psdg = psdg_ctx.__enter__()
                dgTb = [psdg.tile([P, N], F32, tag=f"dgT{dt}", name=f"dgT{dt}") for dt in range(DT)]
                dgT8 = work.tile([P, DT, N], BF16, tag="dgT8", name="dgT8")
                last_step = step == steps - 1
                if with_ar:
                    arin = drp.tile([D, N], BF16, tag="arin")
                    arout = drp.tile([D, N], BF16, tag="arout")
                # dt-outer: each d-chunk's full accumulation (attention +
                # hopfield) finishes early, evacuates immediately and goes
                # straight out to the AllReduce staging buffer, so the
                # collective fires as soon as the last bank stops.
                for dt in range(DT):
                    first = True
                    for et in range(ET):
                        for d_t, w_t in ((dqTst, wqt_sb), (dkTst, wkt_sb)):
                            nc.tensor.matmul(
                                dgTb[dt][:], w_t[:, et, dt * P : (dt + 1) * P],
                                d_t[:, et, :], start=first, stop=False,
                            )
                            first = False
                    for mt in range(MT):
                        nc.tensor.matmul(
                            dgTb[dt][:], xi_sb[:, mt, dt * P : (dt + 1) * P], RT[:, mt, :],
                            start=False, stop=(mt == MT - 1),
                        )
                    nc.vector.tensor_copy(out=dgT8[:, dt, :], in_=dgTb[dt][:])
                    if with_ar:
                        nc.sync.dma_start(out=arin[dt * P : (dt + 1) * P, :], in_=dgT8[:, dt, :])
                psdg_ctx.__exit__(None, None, None)

                # ======== pair AllReduce on dgT (LN-bwd is linear, so the
                # whole LN-bwd/update/LN-fwd tail runs on the reduced gradient
                # AFTER the collective, pipelined per row-chunk) ========
                if with_ar:
                    nc.gpsimd.collective_compute(
                        "AllReduce", OP.add, replica_groups=REPLICA_GROUPS,
                        ins=[arin.opt()], outs=[arout.opt()],
                    )
                    for nt in range(NT):
                        nc.sync.dma_start(
                            out=dgT8[:, :, nt * P : (nt + 1) * P],
                            in_=arout[:, nt * P : (nt + 1) * P].rearrange("(dt p) n -> p dt n", p=P),
                        )

                # ======== per-chunk tail: transpose-back, LN-bwd, update,
                # next step's LN-fwd ========
                dx = work.tile([P, NT, D], F32, tag="dx")
                dxb = work.tile([P, NT, D], BF16, tag="dxb")
                if not last_step:
                    rstd_n = stats.tile([P, NT], F32, tag="rstd")
                import contextlib as _cl

                for nt in range(NT):
                    # chunk 0's whole chain runs at high priority so the next
                    # step's PE work unblocks as early as possible
                    prio = tc.high_priority() if nt == 0 else _cl.nullcontext()
                    with prio:
                        ms = stats.tile([P, 4], F32, tag="ms")
                        pt = ps.tile([P, 512], BF16, tag="ps")
                        for dt in range(4):
                            nc.tensor.transpose(pt[:, dt * P : (dt + 1) * P], dgT8[:, dt, nt * P : (nt + 1) * P], ident_b[:])
                        nc.vector.scalar_tensor_tensor(
                            out=dx[:, nt, 0:512], in0=pt[:], scalar=0.0, in1=xhat[:, nt, 0:512].bitcast(F32),
                            op0=OP.bypass, op1=OP.bypass, accum_out=ms[:, 0:1],
                        )
                        pt2 = ps.tile([P, 512], BF16, tag="ps")
                        for dt in range(4, DT):
                            nc.tensor.transpose(pt2[:, (dt - 4) * P : (dt - 3) * P], dgT8[:, dt, nt * P : (nt + 1) * P], ident_b[:])
                        nc.vector.scalar_tensor_tensor(
                            out=dx[:, nt, 512:768], in0=pt2[:, :256], scalar=0.0, in1=xhat[:, nt, 512:768].bitcast(F32),
                            op0=OP.bypass, op1=OP.bypass, accum_out=ms[:, 1:2],
                        )
                        # u2 = sum(dg*xhat), m1 = sum(dg)
                        prodA = scr.tile([P, D], F32, tag="prodA")
                        nc.vector.scalar_tensor_tensor(
                            out=prodA[:], in0=dx[:, nt, :], scalar=1.0, in1=f_(xhat[:, nt, :]),
                            op0=OP.mult, op1=OP.mult, accum_out=ms[:, 2:3],
                        )
                        nc.vector.tensor_tensor(out=ms[:, 3:4], in0=ms[:, 0:1], in1=ms[:, 1:2], op=OP.add)
                        ss = stats.tile([P, 2], F32, tag="ss")
                        nc.vector.tensor_scalar_mul(out=ss[:], in0=ms[:, 2:4], scalar1=1.0 / D)
                        # dxb = dg - xhat*<dg xhat> - <dg> (rstd folded into update)
                        nc.vector.ln_bwd_dx(
                            out=dxb[:, nt, :], dy=dx[:, nt, :], x_hat=f_(xhat[:, nt, :]),
                            mean_dyx=ss[:, 0:1], mean_dy=ss[:, 1:2], scale=1.0,
                        )
                        # x += (alpha*rstd) * dxb; accum gives sum(x_new) free
                        arr = stats.tile([P, 1], F32, tag="arr")
                        nc.vector.tensor_scalar_mul(out=arr[:], in0=rstd[:, nt : nt + 1], scalar1=ALPHA)
                        sums = stats.tile([P, 2], F32, tag="sums")
                        nc.vector.scalar_tensor_tensor(
                            out=x_sb[:, nt, :], in0=dxb[:, nt, :], scalar=arr[:], in1=x_sb[:, nt, :],
                            op0=OP.mult, op1=OP.add,
                            accum_out=sums[:, 0:1] if not last_step else None,
                        )
                        if not last_step:
                            # sum(x_new^2) on the (tail-idle) Scalar engine
                            xsq = scr.tile([P, D], BF16, tag="xsq")
                            nc.scalar.activation(
                                out=xsq[:], in_=x_sb[:, nt, :], func=AF.Square,
                                accum_out=sums[:, 1:2],
                            )
                            mrow = stats.tile([P, 2], F32, tag="mrow")
                            nc.vector.tensor_scalar_mul(out=mrow[:], in0=sums[:], scalar1=1.0 / D)
                            msq = stats.tile([P, 1], F32, tag="msq")
                            nc.vector.tensor_tensor(out=msq[:], in0=mrow[:, 0:1], in1=mrow[:, 0:1], op=OP.mult)
                            varc = stats.tile([P, 1], F32, tag="varc")
                            nc.vector.tensor_tensor(out=varc[:], in0=mrow[:, 1:2], in1=msq[:], op=OP.subtract)
                            ln_rstd(nt, varc[:], rstd_n)
                            ln_xhat(nt, mrow[:, 0:1], rstd_n, xhat)
                if not last_step:
                    rstd = rstd_n

            for nt in range(NT):
                nc.sync.dma_start(out=x_out[nt * P : (nt + 1) * P, :], in_=x_sb[:, nt, :])

    nc.compile()
    return nc


def _prep_inputs(x, gamma, delta, Wq, Wk, xi):
    """Build the 8 per-core input dicts (host-side sharding + weight folding)."""
    assert np.allclose(delta, 0.0), "kernel requires delta == 0"
    beta_sqrt = np.float32(1.0 / np.sqrt(np.sqrt(np.float32(HD))))
    g = gamma.astype(np.float32)
    import ml_dtypes

    bf = ml_dtypes.bfloat16
    in_maps = []
    for c in range(8):
        b, j = c // 2, c % 2
        hs = slice(j * HL, (j + 1) * HL)
        wq_l = (Wq[hs] * g[None, :, None]).transpose(1, 0, 2).reshape(D, EW)
        wk_l = (Wk[hs] * g[None, :, None]).transpose(1, 0, 2).reshape(D, EW)
        wqt_l = (Wq[hs] * g[None, :, None]).transpose(0, 2, 1).reshape(EW, D)
        wkt_l = (Wk[hs] * g[None, :, None]).transpose(0, 2, 1).reshape(EW, D)
        xi_l = xi[j * ML : (j + 1) * ML] * g[None, :]
        in_maps.append(
            {
                "x": np.ascontiguousarray(x[b]),
                "wq": np.ascontiguousarray(wq_l * beta_sqrt).astype(bf),
                "wk": np.ascontiguousarray(wk_l * beta_sqrt).astype(bf),
                "wqt": np.ascontiguousarray(wqt_l / beta_sqrt).astype(bf),
                "wkt": np.ascontiguousarray(wkt_l / beta_sqrt).astype(bf),
                "xi": np.ascontiguousarray(xi_l).astype(bf),
                "xit": np.ascontiguousarray(xi_l.T).astype(bf),
            }
        )
    return in_maps


_NC_CACHE = {}


def _get_nc(steps=STEPS, with_ar=True):
    key = (steps, with_ar)
    if key not in _NC_CACHE:
        _NC_CACHE[key] = build_kernel(steps, with_ar)
    return _NC_CACHE[key]


def kernel(x, gamma, delta, Wq, Wk, xi):
    from concourse.bass_utils import run_bass_kernel_spmd

    x = np.asarray(x, dtype=np.float32)
    in_maps = _prep_inputs(
        x,
        np.asarray(gamma, np.float32),
        np.asarray(delta, np.float32),
        np.asarray(Wq, np.float32),
        np.asarray(Wk, np.float32),
        np.asarray(xi, np.float32),
    )
    nc = _get_nc()
    res = run_bass_kernel_spmd(nc, in_maps, list(range(8)))
    out = np.stack([res.results[2 * b]["x_out"] for b in range(B)], axis=0)
    return out.astype(np.float32)


# revision 31
# speedup vs baseline: 1.0162x; 1.0162x over previous
"""Energy Transformer descent kernel for 8 Trainium2 NeuronCores (v2).

Problem: 12 steps of gradient descent on
  E(x) = -(1/beta) sum logsumexp(beta q k^T) - 0.5 sum relu(g xi^T)^2,
  g = LayerNorm(x; gamma, delta), q = g Wq_h, k = g Wk_h.

Sharding: data-parallel over batch B=4 -> core pairs (2b, 2b+1); within a
pair, core j takes attention heads j*6..j*6+5 and Hopfield memories
xi[j*1536:(j+1)*1536].  Both energy terms contribute additively to dE/dx
and LayerNorm-backward is linear in the upstream gradient, so each core
computes a partial dx and a pairwise AllReduce produces the full step.

v2 changes vs the baseline:
- P^T is never formed by PE transposes.  Instead E = exp(S) and
  ET = exp(S^T) are both computed by matmul (S^T costs one extra C=64
  matmul per chunk), kept UNnormalized; the softmax 1/Z is folded
  per-partition into q (for dK^T) and applied per-column to dQ^T via a
  row-replicated 1/Z tile built with tiny PE column-transposes + one
  gpsimd partition_broadcast.
- Hopfield h-pass is interleaved with the attention heads (PE filler
  during softmax waits); xi/xi^T live resident in SBUF.
- dgT accumulation is d-chunk-outer so each PSUM bank evacuates as soon
  as its chain stops; the tail transposes overlap remaining banks.
- Step tail split around ONE AllReduce: pass 1 (LN-bwd via fused
  ln_bwd_dx, rstd folded into the update scale) is AR-independent;
  pass 2 fuses the x update with next step's LN stats (update accum =
  sum(x), ACT Square accum = sum(x^2), DVE-Newton rsqrt - no ACT table
  switches anywhere), chunk 0 at high priority so the next step's PE
  work restarts right after the AllReduce.

Host-side preprocessing folds gamma and the attention scale into the
weights (delta must be zero, which the problem guarantees).
"""

import numpy as np

import concourse.bass as bass
import concourse.tile as tile
from concourse import bacc, mybir

STEPS = 12
ALPHA = 0.125
EPS = 1e-5
B, N, D, H, HD, M = 4, 512, 768, 12, 64, 3072
P = 128
NT = N // P  # 4 row chunks
DT = D // P  # 6 embed chunks
HL = H // 2  # heads per core
EW = HL * HD  # 384 local head width
ET = EW // P  # 3 stacked head-pair chunks
ML = M // 2  # memories per core
MT = ML // P  # 12 memory chunks
F32 = mybir.dt.float32
F32R = mybir.dt.float32r
BF16 = mybir.dt.bfloat16
AF = mybir.ActivationFunctionType
OP = mybir.AluOpType

REPLICA_GROUPS = [[0, 1], [2, 3], [4, 5], [6, 7]]


def f_(ap):
    return ap.bitcast(F32)


def build_kernel(steps=STEPS, with_ar=True):
    nc = bacc.Bacc("TRN2", target_bir_lowering=False, debug=False, num_devices=8)

    x_in = nc.declare_dram_parameter("x", [N, D], F32, isOutput=False)
    wq_d = nc.declare_dram_parameter("wq", [D, EW], BF16, isOutput=False)
    wk_d = nc.declare_dram_parameter("wk", [D, EW], BF16, isOutput=False)
    wqt_d = nc.declare_dram_parameter("wqt", [EW, D], BF16, isOutput=False)
    wkt_d = nc.declare_dram_parameter("wkt", [EW, D], BF16, isOutput=False)
    xi_d = nc.declare_dram_parameter("xi", [ML, D], BF16, isOutput=False)
    xit_d = nc.declare_dram_parameter("xit", [D, ML], BF16, isOutput=False)
    x_out = nc.declare_dram_parameter("x_out", [N, D], F32, isOutput=True)

    with tile.TileContext(nc) as tc:
        import contextlib

        with contextlib.ExitStack() as ctx:
            consts = ctx.enter_context(tc.tile_pool(name="consts", bufs=1))
            work = ctx.enter_context(tc.tile_pool(name="work", bufs=1))
            attp = ctx.enter_context(tc.tile_pool(name="attp", bufs=2))
            stats = ctx.enter_context(tc.tile_pool(name="stats", bufs=4))
            rtp = ctx.enter_context(tc.tile_pool(name="rtp", bufs=1))
            scr = ctx.enter_context(tc.tile_pool(name="scr", bufs=2))
            ps = ctx.enter_context(tc.tile_pool(name="ps", bufs=2, space="PSUM"))
            drp = ctx.enter_context(tc.tile_pool(name="drp", bufs=2, space="DRAM"))

            # ---- resident tensors ----
            wq_sb = consts.tile([P, DT, EW], BF16)
            nc.sync.dma_start(out=wq_sb[:], in_=wq_d.rearrange("(dt p) e -> p dt e", p=P))
            wk_sb = consts.tile([P, DT, EW], BF16)
            nc.sync.dma_start(out=wk_sb[:], in_=wk_d.rearrange("(dt p) e -> p dt e", p=P))
            wqt_sb = consts.tile([P, ET, D], BF16)
            nc.sync.dma_start(out=wqt_sb[:], in_=wqt_d.rearrange("(et p) d -> p et d", p=P))
            wkt_sb = consts.tile([P, ET, D], BF16)
            nc.sync.dma_start(out=wkt_sb[:], in_=wkt_d.rearrange("(et p) d -> p et d", p=P))
            x_sb = consts.tile([P, NT, D], F32)
            nc.sync.dma_start(out=x_sb[:], in_=x_in.rearrange("(nt p) d -> p nt d", p=P))
            xit_sb = consts.tile([P, DT, ML], BF16)
            nc.sync.dma_start(out=xit_sb[:], in_=xit_d.rearrange("(dt p) m -> p dt m", p=P))
            xi_sb = consts.tile([P, MT, D], BF16)
            nc.sync.dma_start(out=xi_sb[:], in_=xi_d.rearrange("(mt p) d -> p mt d", p=P))

            from concourse.masks import make_identity

            ident_f = consts.tile([P, P], F32)
            make_identity(nc, ident_f[:])
            ident = consts.tile([P, P], F32R)
            nc.vector.tensor_copy(out=ident[:], in_=ident_f[:])
            ident_b = consts.tile([P, P], BF16)
            nc.vector.tensor_copy(out=ident_b[:], in_=ident_f[:])
            eps_t = consts.tile([P, 1], F32)
            nc.vector.memset(eps_t[:], EPS)

            def ln_stats(nt, mu_t, var_t):
                """bn stats for row-chunk nt -> mean, var columns."""
                xt = x_sb[:, nt, :]
                st = stats.tile([P, 3, 6], F32, tag="bnst")
                xg = xt.rearrange("p (g s) -> p g s", s=256)
                for gs in range(3):
                    nc.vector.bn_stats(out=st[:, gs, :], in_=xg[:, gs, :])
                mv = stats.tile([P, 2], F32, tag="mv")
                nc.vector.bn_aggr(out=mv[:], in_=st[:])
                nc.vector.tensor_copy(out=mu_t[:, nt : nt + 1], in_=mv[:, 0:1])
                nc.vector.tensor_copy(out=var_t[:, nt : nt + 1], in_=mv[:, 1:2])

            def ln_rstd(nt, var_ap, rstd_t):
                """rstd = 1/sqrt(var+eps): linear seed 1.5-0.5(v+eps) + 2
                Newton steps (var stays in [0.8, 1.25] here; no ACT table)."""
                rr = rstd_t[:, nt : nt + 1]
                vh = stats.tile([P, 1], F32, tag="vh")
                nc.vector.tensor_scalar(
                    out=vh[:], in0=var_ap, scalar1=-0.5, scalar2=-0.5 * EPS,
                    op0=OP.mult, op1=OP.add,
                )
                nc.vector.tensor_scalar_add(out=rr, in0=vh[:], scalar1=1.5)
                for _ in range(2):
                    y2 = stats.tile([P, 1], F32, tag="y2")
                    nc.vector.tensor_tensor(out=y2[:], in0=rr, in1=rr, op=OP.mult)
                    nc.vector.tensor_scalar(
                        out=y2[:], in0=y2[:], scalar1=vh[:], scalar2=1.5,
                        op0=OP.mult, op1=OP.add,
                    )
                    nc.vector.tensor_tensor(out=rr, in0=rr, in1=y2[:], op=OP.mult)

            def ln_xhat(nt, mu_ap, rstd_t, xhat_t):
                rr = rstd_t[:, nt : nt + 1]
                nmu = stats.tile([P, 1], F32, tag="nmu")
                nc.vector.scalar_tensor_tensor(
                    out=nmu[:], in0=mu_ap, scalar=-1.0, in1=rr, op0=OP.mult, op1=OP.mult,
                )
                nc.scalar.activation(
                    out=xhat_t[:, nt, :], in_=x_sb[:, nt, :], func=AF.Identity, scale=rr, bias=nmu[:],
                )

            # LN-fwd for step 0
            xhat = work.tile([P, NT, D], F32R, tag="xhat")
            rstd = stats.tile([P, NT], F32, tag="rstd")
            mu0 = stats.tile([P, NT], F32, tag="mu")
            var0 = stats.tile([P, NT], F32, tag="var")
            for nt in range(NT):
                ln_stats(nt, mu0, var0)
                ln_rstd(nt, var0[:, nt : nt + 1], rstd)
                ln_xhat(nt, mu0[:, nt : nt + 1], rstd, xhat)

            for step in range(steps):
                # ======== gT = xhat^T [d-part, n-free] ========
                psw_ctx = tc.tile_pool(name="psw", bufs=6, space="PSUM")
                psw = psw_ctx.__enter__()
                gT = work.tile([P, DT, N], BF16, tag="gT")
                # nt-outer so each row-chunk's transposes unblock right after
                # that chunk's xhat lands (pipelines across the step boundary)
                for nt in range(NT):
                    pa = psw.tile([P, 512], F32R, tag="psw")
                    for dt in range(4):
                        nc.tensor.transpose(pa[:, dt * P : (dt + 1) * P], xhat[:, nt, dt * P : (dt + 1) * P], ident[:])
                    pb = psw.tile([P, 512], F32R, tag="psw")
                    for dt in range(4, DT):
                        nc.tensor.transpose(pb[:, (dt - 4) * P : (dt - 3) * P], xhat[:, nt, dt * P : (dt + 1) * P], ident[:])
                    nc.vector.tensor_copy(out=gT[:, 0:4, nt * P : (nt + 1) * P], in_=pa[:].rearrange("p (dt c) -> p dt c", c=P))
                    nc.vector.tensor_copy(out=gT[:, 4:6, nt * P : (nt + 1) * P], in_=pb[:, 0:256].rearrange("p (dt c) -> p dt c", c=P))

                # ======== projections q, k [n-part, e] ========
                q = work.tile([P, NT, EW], BF16, tag="q")
                k = work.tile([P, NT, EW], BF16, tag="k")
                for nt in range(NT):
                    ppq = psw.tile([P, 512], F32, tag="psw")
                    ppk = psw.tile([P, 512], F32, tag="psw")
                    for dt in range(DT):
                        lh = gT[:, dt, nt * P : (nt + 1) * P]
                        nc.tensor.matmul(ppq[:, :EW], lh, wq_sb[:, dt, :], start=(dt == 0), stop=(dt == DT - 1))
                        nc.tensor.matmul(ppk[:, :EW], lh, wk_sb[:, dt, :], start=(dt == 0), stop=(dt == DT - 1))
                    nc.vector.tensor_copy(out=q[:, nt, :], in_=ppq[:, :EW])
                    nc.vector.tensor_copy(out=k[:, nt, :], in_=ppk[:, :EW])
                # qT, kT [e-part, n]
                qT = work.tile([P, ET, N], BF16, tag="qT")
                kT = work.tile([P, ET, N], BF16, tag="kT")
                for dst, srct in ((qT, q), (kT, k)):
                    for et in range(ET):
                        pp = psw.tile([P, 512], BF16, tag="psw")
                        for nt in range(NT):
                            nc.tensor.transpose(
                                pp[:, nt * P : (nt + 1) * P],
                                srct[:, nt, et * P : (et + 1) * P], ident_b[:],
                            )
                        nc.vector.tensor_copy(out=dst[:, et, :], in_=pp[:])

                # ======== attention heads (+ interleaved hopfield h-pass) ========
                RT = rtp.tile([P, MT, N], BF16, tag="RT")
                dqTst = work.tile([P, ET, N], BF16, tag="dqTst")
                dkTst = work.tile([P, ET, N], BF16, tag="dkTst")
                for h in range(HL):
                    et, eo = h // 2, (h % 2) * HD
                    qTh = qT[eo : eo + HD, et, :]
                    kTh = kT[eo : eo + HD, et, :]
                    # E = exp(S) rows, unnormalized; Z row-sums
                    Eu = attp.tile([P, NT, N], BF16, tag="Eu")
                    ETu = attp.tile([P, NT, N], BF16, tag="ETu")
                    zz = stats.tile([P, NT], F32, tag="zz")
                    for nt in range(NT):
                        sc = psw.tile([P, 512], F32, tag="psw")
                        nc.tensor.matmul(
                            sc[:], qT[eo : eo + HD, et, nt * P : (nt + 1) * P], kTh,
                            start=True, stop=True,
                        )
                        nc.scalar.activation(
                            out=Eu[:, nt, :], in_=sc[:], func=AF.Exp, bias=0.0, scale=1.0,
                            accum_out=zz[:, nt : nt + 1],
                        )
                    rz = stats.tile([P, NT], F32, tag="rz")
                    nc.vector.reciprocal(out=rz[:], in_=zz[:])
                    # row-replicated 1/Z tile [128, N]: column transposes to [1, N],
                    # then one gpsimd partition broadcast
                    rzps = psw.tile([P, 512], F32, tag="psw")
                    for c in range(NT):
                        nc.tensor.transpose(rzps[0:1, c * P : (c + 1) * P], rz[:, c : c + 1], ident_f[:])
                    rzT = stats.tile([1, N], F32, tag="rzT")
                    nc.vector.tensor_copy(out=rzT[:], in_=rzps[0:1, 0:N])
                    rzb = stats.tile([P, N], F32, tag="rzb")
                    nc.gpsimd.partition_broadcast(rzb[:, :], rzT[0:1, :])
                    # ET = exp(S^T), unnormalized
                    for mt in range(NT):
                        scT = psw.tile([P, 512], F32, tag="psw")
                        nc.tensor.matmul(
                            scT[:], kT[eo : eo + HD, et, mt * P : (mt + 1) * P], qTh,
                            start=True, stop=True,
                        )
                        nc.scalar.activation(out=ETu[:, mt, :], in_=scT[:], func=AF.Exp)
                    # dqT_h = (sum_mt k_h[mt]^T ET[mt]) * rzb
                    pp = psw.tile([P, 512], F32, tag="psw")
                    for mt in range(NT):
                        nc.tensor.matmul(
                            pp[:HD, :], k[:, mt, h * HD : (h + 1) * HD], ETu[:, mt, :],
                            start=(mt == 0), stop=(mt == NT - 1),
                        )
                    nc.vector.tensor_tensor(
                        out=dqTst[eo : eo + HD, et, :], in0=pp[:HD, :], in1=rzb[0:HD, :], op=OP.mult,
                    )
                    # dkT_h = sum_nt (q_h[nt] * rz)^T E[nt]
                    qs = scr.tile([P, NT, HD], BF16, tag="qs")
                    for nt in range(NT):
                        nc.vector.tensor_scalar_mul(
                            out=qs[:, nt, :], in0=q[:, nt, h * HD : (h + 1) * HD],
                            scalar1=rz[:, nt : nt + 1],
                        )
                    pp2 = psw.tile([P, 512], F32, tag="psw")
                    for nt in range(NT):
                        nc.tensor.matmul(
                            pp2[:HD, :], qs[:, nt, :], Eu[:, nt, :],
                            start=(nt == 0), stop=(nt == NT - 1),
                        )
                    nc.vector.tensor_copy(out=dkTst[eo : eo + HD, et, :], in_=pp2[:HD, :])
                    # hopfield h-pass chunks riding along with this head
                    for mt in (2 * h, 2 * h + 1):
                        hp = ps.tile([P, 512], F32, tag="ps")
                        for dt in range(DT):
                            nc.tensor.matmul(
                                hp[:], xit_sb[:, dt, mt * P : (mt + 1) * P], gT[:, dt, :],
                                start=(dt == 0), stop=(dt == DT - 1),
                            )
                        nc.scalar.activation(out=RT[:, mt, :], in_=hp[:], func=AF.Relu)

                psw_ctx.__exit__(None, None, None)
                # ======== dg accumulation in PSUM, transposed [d-chunk, n] ========
                psdg_ctx = tc.tile_pool(name="psdg", bufs=1, space="PSUM")
                psdg = psdg_ctx.__enter__()
                dgTb = [psdg.tile([P, N], F32, tag=f"dgT{dt}", name=f"dgT{dt}") for dt in range(DT)]
                dgTs = work.tile([P, DT, N], F32R, tag="dgTs")
                # dt-outer: each d-chunk's full accumulation (attention +
                # hopfield) finishes early and evacuates immediately, so the
                # tail transposes overlap the remaining banks' matmuls
                for dt in range(DT):
                    first = True
                    for et in range(ET):
                        for d_t, w_t in ((dqTst, wqt_sb), (dkTst, wkt_sb)):
                            nc.tensor.matmul(
                                dgTb[dt][:], w_t[:, et, dt * P : (dt + 1) * P],
                                d_t[:, et, :], start=first, stop=False,
                            )
                            first = False
                    for mt in range(MT):
                        nc.tensor.matmul(
                            dgTb[dt][:], xi_sb[:, mt, dt * P : (dt + 1) * P], RT[:, mt, :],
                            start=False, stop=(mt == MT - 1),
                        )
                    nc.vector.tensor_copy(out=dgTs[:, dt, :], in_=dgTb[dt][:])
                psdg_ctx.__exit__(None, None, None)

                # ======== tail: pass 1 (AR-independent) per row-chunk ========
                dx = work.tile([P, NT, D], F32, tag="dx")
                dxb = work.tile([P, NT, D], BF16, tag="dxb")
                last_step = step == steps - 1
                if with_ar:
                    arin = drp.tile([N, D], BF16, tag="arin")
                    arout = drp.tile([N, D], BF16, tag="arout")
                for nt in range(NT):
                    ms = stats.tile([P, 4], F32, tag="ms")
                    pt = ps.tile([P, 512], F32R, tag="ps")
                    for dt in range(4):
                        nc.tensor.transpose(pt[:, dt * P : (dt + 1) * P], dgTs[:, dt, nt * P : (nt + 1) * P], ident[:])
                    nc.vector.scalar_tensor_tensor(
                        out=dx[:, nt, 0:512], in0=f_(pt[:]), scalar=0.0, in1=xhat[:, nt, 0:512].bitcast(F32),
                        op0=OP.bypass, op1=OP.bypass, accum_out=ms[:, 0:1],
                    )
                    pt2 = ps.tile([P, 512], F32R, tag="ps")
                    for dt in range(4, DT):
                        nc.tensor.transpose(pt2[:, (dt - 4) * P : (dt - 3) * P], dgTs[:, dt, nt * P : (nt + 1) * P], ident[:])
                    nc.vector.scalar_tensor_tensor(
                        out=dx[:, nt, 512:768], in0=f_(pt2[:, :256]), scalar=0.0, in1=xhat[:, nt, 512:768].bitcast(F32),
                        op0=OP.bypass, op1=OP.bypass, accum_out=ms[:, 1:2],
                    )
                    # u2 = sum(dg*xhat), m1 = sum(dg)
                    prodA = scr.tile([P, D], F32, tag="prodA")
                    nc.vector.scalar_tensor_tensor(
                        out=prodA[:], in0=dx[:, nt, :], scalar=1.0, in1=f_(xhat[:, nt, :]),
                        op0=OP.mult, op1=OP.mult, accum_out=ms[:, 2:3],
                    )
                    nc.vector.tensor_tensor(out=ms[:, 3:4], in0=ms[:, 0:1], in1=ms[:, 1:2], op=OP.add)
                    ss = stats.tile([P, 2], F32, tag="ss")
                    nc.vector.tensor_scalar_mul(out=ss[:], in0=ms[:, 2:4], scalar1=1.0 / D)
                    # dxb = dg - xhat*<dg xhat> - <dg> (rstd folded into update)
                    nc.vector.ln_bwd_dx(
                        out=dxb[:, nt, :], dy=dx[:, nt, :], x_hat=f_(xhat[:, nt, :]),
                        mean_dyx=ss[:, 0:1], mean_dy=ss[:, 1:2], scale=1.0,
                    )
                    if with_ar:
                        nc.sync.dma_start(out=arin[nt * P : (nt + 1) * P, :], in_=dxb[:, nt, :])
                # ======== single pair AllReduce ========
                if with_ar:
                    nc.gpsimd.collective_compute(
                        "AllReduce", OP.add, replica_groups=REPLICA_GROUPS,
                        ins=[arin.opt()], outs=[arout.opt()],
                    )
                    for nt in range(NT):
                        nc.sync.dma_start(out=dxb[:, nt, :], in_=arout[nt * P : (nt + 1) * P, :])
                # ======== tail pass 2: update (+fused stats) + next LN-fwd ========
                if not last_step:
                    rstd_n = stats.tile([P, NT], F32, tag="rstd")
                import contextlib as _cl

                for nt in range(NT):
                    # chunk 0's whole chain runs at high priority so the next
                    # step's PE work unblocks as early as possible
                    prio = tc.high_priority() if nt == 0 else _cl.nullcontext()
                    with prio:
                        # x += (alpha*rstd) * dxb; accum gives sum(x_new) free
                        arr = stats.tile([P, 1], F32, tag="arr")
                        nc.vector.tensor_scalar_mul(out=arr[:], in0=rstd[:, nt : nt + 1], scalar1=ALPHA)
                        sums = stats.tile([P, 2], F32, tag="sums")
                        nc.vector.scalar_tensor_tensor(
                            out=x_sb[:, nt, :], in0=dxb[:, nt, :], scalar=arr[:], in1=x_sb[:, nt, :],
                            op0=OP.mult, op1=OP.add,
                            accum_out=sums[:, 0:1] if not last_step else None,
                        )
                        if not last_step:
                            # sum(x_new^2) on the (tail-idle) Scalar engine
                            xsq = scr.tile([P, D], BF16, tag="xsq")
                            nc.scalar.activation(
                                out=xsq[:], in_=x_sb[:, nt, :], func=AF.Square,
                                accum_out=sums[:, 1:2],
                            )
                            mrow = stats.tile([P, 2], F32, tag="mrow")
                            nc.vector.tensor_scalar_mul(out=mrow[:], in0=sums[:], scalar1=1.0 / D)
                            msq = stats.tile([P, 1], F32, tag="msq")
                            nc.vector.tensor_tensor(out=msq[:], in0=mrow[:, 0:1], in1=mrow[:, 0:1], op=OP.mult)
                            varc = stats.tile([P, 1], F32, tag="varc")
                            nc.vector.tensor_tensor(out=varc[:], in0=mrow[:, 1:2], in1=msq[:], op=OP.subtract)
                            ln_rstd(nt, varc[:], rstd_n)
                            ln_xhat(nt, mrow[:, 0:1], rstd_n, xhat)
                if not last_step:
                    rstd = rstd_n

            for nt in range(NT):
                nc.sync.dma_start(out=x_out[nt * P : (nt + 1) * P, :], in_=x_sb[:, nt, :])

    nc.compile()
    return nc


def _prep_inputs(x, gamma, delta, Wq, Wk, xi):
    """Build the 8 per-core input dicts (host-side sharding + weight folding)."""
    assert np.allclose(delta, 0.0), "kernel requires delta == 0"
    beta_sqrt = np.float32(1.0 / np.sqrt(np.sqrt(np.float32(HD))))
    g = gamma.astype(np.float32)
    import ml_dtypes

    bf = ml_dtypes.bfloat16
    in_maps = []
    for c in range(8):
        b, j = c // 2, c % 2
        hs = slice(j * HL, (j + 1) * HL)
        wq_l = (Wq[hs] * g[None, :, None]).transpose(1, 0, 2).reshape(D, EW)
        wk_l = (Wk[hs] * g[None, :, None]).transpose(1, 0, 2).reshape(D, EW)
        wqt_l = (Wq[hs] * g[None, :, None]).transpose(0, 2, 1).reshape(EW, D)
        wkt_l = (Wk[hs] * g[None, :, None]).transpose(0, 2, 1).reshape(EW, D)
        xi_l = xi[j * ML : (j + 1) * ML] * g[None, :]
        in_maps.append(
            {
                "x": np.ascontiguousarray(x[b]),
                "wq": np.ascontiguousarray(wq_l * beta_sqrt).astype(bf),
                "wk": np.ascontiguousarray(wk_l * beta_sqrt).astype(bf),
                "wqt": np.ascontiguousarray(wqt_l / beta_sqrt).astype(bf),
                "wkt": np.ascontiguousarray(wkt_l / beta_sqrt).astype(bf),
                "xi": np.ascontiguousarray(xi_l).astype(bf),
                "xit": np.ascontiguousarray(xi_l.T).astype(bf),
            }
        )
    return in_maps


_NC_CACHE = {}


def _get_nc(steps=STEPS, with_ar=True):
    key = (steps, with_ar)
    if key not in _NC_CACHE:
        _NC_CACHE[key] = build_kernel(steps, with_ar)
    return _NC_CACHE[key]


def kernel(x, gamma, delta, Wq, Wk, xi):
    from concourse.bass_utils import run_bass_kernel_spmd

    x = np.asarray(x, dtype=np.float32)
    in_maps = _prep_inputs(
        x,
        np.asarray(gamma, np.float32),
        np.asarray(delta, np.float32),
        np.asarray(Wq, np.float32),
        np.asarray(Wk, np.float32),
        np.asarray(xi, np.float32),
    )
    nc = _get_nc()
    res = run_bass_kernel_spmd(nc, in_maps, list(range(8)))
    out = np.stack([res.results[2 * b]["x_out"] for b in range(B)], axis=0)
    return out.astype(np.float32)


# revision 32
# speedup vs baseline: 1.0168x; 1.0007x over previous
"""Energy Transformer descent kernel for 8 Trainium2 NeuronCores (v2).

Problem: 12 steps of gradient descent on
  E(x) = -(1/beta) sum logsumexp(beta q k^T) - 0.5 sum relu(g xi^T)^2,
  g = LayerNorm(x; gamma, delta), q = g Wq_h, k = g Wk_h.

Sharding: data-parallel over batch B=4 -> core pairs (2b, 2b+1); within a
pair, core j takes attention heads j*6..j*6+5 and Hopfield memories
xi[j*1536:(j+1)*1536].  Both energy terms contribute additively to dE/dx
and LayerNorm-backward is linear in the upstream gradient, so each core
computes a partial dx and a pairwise AllReduce produces the full step.

v2 changes vs the baseline:
- P^T is never formed by PE transposes.  Instead E = exp(S) and
  ET = exp(S^T) are both computed by matmul (S^T costs one extra C=64
  matmul per chunk), kept UNnormalized; the softmax 1/Z is folded
  per-partition into q (for dK^T) and applied per-column to dQ^T via a
  row-replicated 1/Z tile built with tiny PE column-transposes + one
  gpsimd partition_broadcast.
- Hopfield h-pass is interleaved with the attention heads (PE filler
  during softmax waits); xi/xi^T live resident in SBUF.
- dgT accumulation is d-chunk-outer so each PSUM bank evacuates as soon
  as its chain stops; the tail transposes overlap remaining banks.
- Step tail split around ONE AllReduce: pass 1 (LN-bwd via fused
  ln_bwd_dx, rstd folded into the update scale) is AR-independent;
  pass 2 fuses the x update with next step's LN stats (update accum =
  sum(x), ACT Square accum = sum(x^2), DVE-Newton rsqrt - no ACT table
  switches anywhere), chunk 0 at high priority so the next step's PE
  work restarts right after the AllReduce.

Host-side preprocessing folds gamma and the attention scale into the
weights (delta must be zero, which the problem guarantees).
"""

import numpy as np

import concourse.bass as bass
import concourse.tile as tile
from concourse import bacc, mybir

STEPS = 12
ALPHA = 0.125
EPS = 1e-5
B, N, D, H, HD, M = 4, 512, 768, 12, 64, 3072
P = 128
NT = N // P  # 4 row chunks
DT = D // P  # 6 embed chunks
HL = H // 2  # heads per core
EW = HL * HD  # 384 local head width
ET = EW // P  # 3 stacked head-pair chunks
ML = M // 2  # memories per core
MT = ML // P  # 12 memory chunks
F32 = mybir.dt.float32
F32R = mybir.dt.float32r
BF16 = mybir.dt.bfloat16
AF = mybir.ActivationFunctionType
OP = mybir.AluOpType

REPLICA_GROUPS = [[0, 1], [2, 3], [4, 5], [6, 7]]


def f_(ap):
    return ap.bitcast(F32)


def build_kernel(steps=STEPS, with_ar=True):
    nc = bacc.Bacc("TRN2", target_bir_lowering=False, debug=False, num_devices=8)

    x_in = nc.declare_dram_parameter("x", [N, D], F32, isOutput=False)
    wq_d = nc.declare_dram_parameter("wq", [D, EW], BF16, isOutput=False)
    wk_d = nc.declare_dram_parameter("wk", [D, EW], BF16, isOutput=False)
    wqt_d = nc.declare_dram_parameter("wqt", [EW, D], BF16, isOutput=False)
    wkt_d = nc.declare_dram_parameter("wkt", [EW, D], BF16, isOutput=False)
    xi_d = nc.declare_dram_parameter("xi", [ML, D], BF16, isOutput=False)
    xit_d = nc.declare_dram_parameter("xit", [D, ML], BF16, isOutput=False)
    x_out = nc.declare_dram_parameter("x_out", [N, D], F32, isOutput=True)

    with tile.TileContext(nc) as tc:
        import contextlib

        with contextlib.ExitStack() as ctx:
            consts = ctx.enter_context(tc.tile_pool(name="consts", bufs=1))
            work = ctx.enter_context(tc.tile_pool(name="work", bufs=1))
            attp = ctx.enter_context(tc.tile_pool(name="attp", bufs=3))
            stats = ctx.enter_context(tc.tile_pool(name="stats", bufs=6))
            rtp = ctx.enter_context(tc.tile_pool(name="rtp", bufs=1))
            scr = ctx.enter_context(tc.tile_pool(name="scr", bufs=2))
            ps = ctx.enter_context(tc.tile_pool(name="ps", bufs=2, space="PSUM"))
            drp = ctx.enter_context(tc.tile_pool(name="drp", bufs=2, space="DRAM"))

            # ---- resident tensors ----
            wq_sb = consts.tile([P, DT, EW], BF16)
            nc.sync.dma_start(out=wq_sb[:], in_=wq_d.rearrange("(dt p) e -> p dt e", p=P))
            wk_sb = consts.tile([P, DT, EW], BF16)
            nc.sync.dma_start(out=wk_sb[:], in_=wk_d.rearrange("(dt p) e -> p dt e", p=P))
            wqt_sb = consts.tile([P, ET, D], BF16)
            nc.sync.dma_start(out=wqt_sb[:], in_=wqt_d.rearrange("(et p) d -> p et d", p=P))
            wkt_sb = consts.tile([P, ET, D], BF16)
            nc.sync.dma_start(out=wkt_sb[:], in_=wkt_d.rearrange("(et p) d -> p et d", p=P))
            x_sb = consts.tile([P, NT, D], F32)
            nc.sync.dma_start(out=x_sb[:], in_=x_in.rearrange("(nt p) d -> p nt d", p=P))
            xit_sb = consts.tile([P, DT, ML], BF16)
            nc.sync.dma_start(out=xit_sb[:], in_=xit_d.rearrange("(dt p) m -> p dt m", p=P))
            xi_sb = consts.tile([P, MT, D], BF16)
            nc.sync.dma_start(out=xi_sb[:], in_=xi_d.rearrange("(mt p) d -> p mt d", p=P))

            from concourse.masks import make_identity

            ident_f = consts.tile([P, P], F32)
            make_identity(nc, ident_f[:])
            ident = consts.tile([P, P], F32R)
            nc.vector.tensor_copy(out=ident[:], in_=ident_f[:])
            ident_b = consts.tile([P, P], BF16)
            nc.vector.tensor_copy(out=ident_b[:], in_=ident_f[:])
            eps_t = consts.tile([P, 1], F32)
            nc.vector.memset(eps_t[:], EPS)

            def ln_stats(nt, mu_t, var_t):
                """bn stats for row-chunk nt -> mean, var columns."""
                xt = x_sb[:, nt, :]
                st = stats.tile([P, 3, 6], F32, tag="bnst")
                xg = xt.rearrange("p (g s) -> p g s", s=256)
                for gs in range(3):
                    nc.vector.bn_stats(out=st[:, gs, :], in_=xg[:, gs, :])
                mv = stats.tile([P, 2], F32, tag="mv")
                nc.vector.bn_aggr(out=mv[:], in_=st[:])
                nc.vector.tensor_copy(out=mu_t[:, nt : nt + 1], in_=mv[:, 0:1])
                nc.vector.tensor_copy(out=var_t[:, nt : nt + 1], in_=mv[:, 1:2])

            def ln_rstd(nt, var_ap, rstd_t):
                """rstd = 1/sqrt(var+eps): linear seed 1.5-0.5(v+eps) + 2
                Newton steps (var stays in [0.8, 1.25] here; no ACT table)."""
                rr = rstd_t[:, nt : nt + 1]
                vh = stats.tile([P, 1], F32, tag="vh")
                nc.vector.tensor_scalar(
                    out=vh[:], in0=var_ap, scalar1=-0.5, scalar2=-0.5 * EPS,
                    op0=OP.mult, op1=OP.add,
                )
                nc.vector.tensor_scalar_add(out=rr, in0=vh[:], scalar1=1.5)
                for _ in range(2):
                    y2 = stats.tile([P, 1], F32, tag="y2")
                    nc.vector.tensor_tensor(out=y2[:], in0=rr, in1=rr, op=OP.mult)
                    nc.vector.tensor_scalar(
                        out=y2[:], in0=y2[:], scalar1=vh[:], scalar2=1.5,
                        op0=OP.mult, op1=OP.add,
                    )
                    nc.vector.tensor_tensor(out=rr, in0=rr, in1=y2[:], op=OP.mult)

            def ln_xhat(nt, mu_ap, rstd_t, xhat_t):
                rr = rstd_t[:, nt : nt + 1]
                nmu = stats.tile([P, 1], F32, tag="nmu")
                nc.vector.scalar_tensor_tensor(
                    out=nmu[:], in0=mu_ap, scalar=-1.0, in1=rr, op0=OP.mult, op1=OP.mult,
                )
                nc.scalar.activation(
                    out=xhat_t[:, nt, :], in_=x_sb[:, nt, :], func=AF.Identity, scale=rr, bias=nmu[:],
                )

            # LN-fwd for step 0
            xhat = work.tile([P, NT, D], F32R, tag="xhat")
            rstd = stats.tile([P, NT], F32, tag="rstd")
            mu0 = stats.tile([P, NT], F32, tag="mu")
            var0 = stats.tile([P, NT], F32, tag="var")
            for nt in range(NT):
                ln_stats(nt, mu0, var0)
                ln_rstd(nt, var0[:, nt : nt + 1], rstd)
                ln_xhat(nt, mu0[:, nt : nt + 1], rstd, xhat)

            for step in range(steps):
                # ======== gT = xhat^T [d-part, n-free] ========
                psw_ctx = tc.tile_pool(name="psw", bufs=6, space="PSUM")
                psw = psw_ctx.__enter__()
                gT = work.tile([P, DT, N], BF16, tag="gT")
                # nt-outer so each row-chunk's transposes unblock right after
                # that chunk's xhat lands (pipelines across the step boundary)
                for nt in range(NT):
                    pa = psw.tile([P, 512], F32R, tag="psw")
                    for dt in range(4):
                        nc.tensor.transpose(pa[:, dt * P : (dt + 1) * P], xhat[:, nt, dt * P : (dt + 1) * P], ident[:])
                    pb = psw.tile([P, 512], F32R, tag="psw")
                    for dt in range(4, DT):
                        nc.tensor.transpose(pb[:, (dt - 4) * P : (dt - 3) * P], xhat[:, nt, dt * P : (dt + 1) * P], ident[:])
                    nc.vector.tensor_copy(out=gT[:, 0:4, nt * P : (nt + 1) * P], in_=pa[:].rearrange("p (dt c) -> p dt c", c=P))
                    nc.vector.tensor_copy(out=gT[:, 4:6, nt * P : (nt + 1) * P], in_=pb[:, 0:256].rearrange("p (dt c) -> p dt c", c=P))

                # ======== projections q, k [n-part, e] ========
                q = work.tile([P, NT, EW], BF16, tag="q")
                k = work.tile([P, NT, EW], BF16, tag="k")
                for nt in range(NT):
                    ppq = psw.tile([P, 512], F32, tag="psw")
                    ppk = psw.tile([P, 512], F32, tag="psw")
                    for dt in range(DT):
                        lh = gT[:, dt, nt * P : (nt + 1) * P]
                        nc.tensor.matmul(ppq[:, :EW], lh, wq_sb[:, dt, :], start=(dt == 0), stop=(dt == DT - 1))
                        nc.tensor.matmul(ppk[:, :EW], lh, wk_sb[:, dt, :], start=(dt == 0), stop=(dt == DT - 1))
                    nc.vector.tensor_copy(out=q[:, nt, :], in_=ppq[:, :EW])
                    nc.vector.tensor_copy(out=k[:, nt, :], in_=ppk[:, :EW])
                # qT, kT [e-part, n]
                qT = work.tile([P, ET, N], BF16, tag="qT")
                kT = work.tile([P, ET, N], BF16, tag="kT")
                for dst, srct in ((qT, q), (kT, k)):
                    for et in range(ET):
                        pp = psw.tile([P, 512], BF16, tag="psw")
                        for nt in range(NT):
                            nc.tensor.transpose(
                                pp[:, nt * P : (nt + 1) * P],
                                srct[:, nt, et * P : (et + 1) * P], ident_b[:],
                            )
                        nc.vector.tensor_copy(out=dst[:, et, :], in_=pp[:])

                # ======== attention heads (+ interleaved hopfield h-pass) ========
                RT = rtp.tile([P, MT, N], BF16, tag="RT")
                dqTst = work.tile([P, ET, N], BF16, tag="dqTst")
                dkTst = work.tile([P, ET, N], BF16, tag="dkTst")
                for h in range(HL):
                    et, eo = h // 2, (h % 2) * HD
                    qTh = qT[eo : eo + HD, et, :]
                    kTh = kT[eo : eo + HD, et, :]
                    # E = exp(S) rows, unnormalized; Z row-sums
                    Eu = attp.tile([P, NT, N], BF16, tag="Eu")
                    ETu = attp.tile([P, NT, N], BF16, tag="ETu")
                    zz = stats.tile([P, NT], F32, tag="zz")
                    for nt in range(NT):
                        sc = psw.tile([P, 512], F32, tag="psw")
                        nc.tensor.matmul(
                            sc[:], qT[eo : eo + HD, et, nt * P : (nt + 1) * P], kTh,
                            start=True, stop=True,
                        )
                        nc.scalar.activation(
                            out=Eu[:, nt, :], in_=sc[:], func=AF.Exp, bias=0.0, scale=1.0,
                            accum_out=zz[:, nt : nt + 1],
                        )
                    rz = stats.tile([P, NT], F32, tag="rz")
                    nc.vector.reciprocal(out=rz[:], in_=zz[:])
                    # row-replicated 1/Z tile [128, N]: column transposes to [1, N],
                    # then one gpsimd partition broadcast
                    rzps = psw.tile([P, 512], F32, tag="psw")
                    for c in range(NT):
                        nc.tensor.transpose(rzps[0:1, c * P : (c + 1) * P], rz[:, c : c + 1], ident_f[:])
                    rzT = stats.tile([1, N], F32, tag="rzT")
                    nc.vector.tensor_copy(out=rzT[:], in_=rzps[0:1, 0:N])
                    rzb = stats.tile([P, N], F32, tag="rzb")
                    nc.gpsimd.partition_broadcast(rzb[:, :], rzT[0:1, :])
                    # ET = exp(S^T), unnormalized
                    for mt in range(NT):
                        scT = psw.tile([P, 512], F32, tag="psw")
                        nc.tensor.matmul(
                            scT[:], kT[eo : eo + HD, et, mt * P : (mt + 1) * P], qTh,
                            start=True, stop=True,
                        )
                        nc.scalar.activation(out=ETu[:, mt, :], in_=scT[:], func=AF.Exp)
                    # dqT_h = (sum_mt k_h[mt]^T ET[mt]) * rzb
                    pp = psw.tile([P, 512], F32, tag="psw")
                    for mt in range(NT):
                        nc.tensor.matmul(
                            pp[:HD, :], k[:, mt, h * HD : (h + 1) * HD], ETu[:, mt, :],
                            start=(mt == 0), stop=(mt == NT - 1),
                        )
                    nc.vector.tensor_tensor(
                        out=dqTst[eo : eo + HD, et, :], in0=pp[:HD, :], in1=rzb[0:HD, :], op=OP.mult,
                    )
                    # dkT_h = sum_nt (q_h[nt] * rz)^T E[nt]
                    qs = scr.tile([P, NT, HD], BF16, tag="qs")
                    for nt in range(NT):
                        nc.vector.tensor_scalar_mul(
                            out=qs[:, nt, :], in0=q[:, nt, h * HD : (h + 1) * HD],
                            scalar1=rz[:, nt : nt + 1],
                        )
                    pp2 = psw.tile([P, 512], F32, tag="psw")
                    for nt in range(NT):
                        nc.tensor.matmul(
                            pp2[:HD, :], qs[:, nt, :], Eu[:, nt, :],
                            start=(nt == 0), stop=(nt == NT - 1),
                        )
                    nc.vector.tensor_copy(out=dkTst[eo : eo + HD, et, :], in_=pp2[:HD, :])
                    # hopfield h-pass chunks riding along with this head
                    for mt in (2 * h, 2 * h + 1):
                        hp = ps.tile([P, 512], F32, tag="ps")
                        for dt in range(DT):
                            nc.tensor.matmul(
                                hp[:], xit_sb[:, dt, mt * P : (mt + 1) * P], gT[:, dt, :],
                                start=(dt == 0), stop=(dt == DT - 1),
                            )
                        nc.scalar.activation(out=RT[:, mt, :], in_=hp[:], func=AF.Relu)

                psw_ctx.__exit__(None, None, None)
                # ======== dg accumulation in PSUM, transposed [d-chunk, n] ========
                psdg_ctx = tc.tile_pool(name="psdg", bufs=1, space="PSUM")
                psdg = psdg_ctx.__enter__()
                dgTb = [psdg.tile([P, N], F32, tag=f"dgT{dt}", name=f"dgT{dt}") for dt in range(DT)]
                dgTs = work.tile([P, DT, N], F32R, tag="dgTs")
                # dt-outer: each d-chunk's full accumulation (attention +
                # hopfield) finishes early and evacuates immediately, so the
                # tail transposes overlap the remaining banks' matmuls
                for dt in range(DT):
                    first = True
                    for et in range(ET):
                        for d_t, w_t in ((dqTst, wqt_sb), (dkTst, wkt_sb)):
                            nc.tensor.matmul(
                                dgTb[dt][:], w_t[:, et, dt * P : (dt + 1) * P],
                                d_t[:, et, :], start=first, stop=False,
                            )
                            first = False
                    for mt in range(MT):
                        nc.tensor.matmul(
                            dgTb[dt][:], xi_sb[:, mt, dt * P : (dt + 1) * P], RT[:, mt, :],
                            start=False, stop=(mt == MT - 1),
                        )
                    nc.vector.tensor_copy(out=dgTs[:, dt, :], in_=dgTb[dt][:])
                psdg_ctx.__exit__(None, None, None)

                # ======== tail: pass 1 (AR-independent) per row-chunk ========
                dx = work.tile([P, NT, D], F32, tag="dx")
                dxb = work.tile([P, NT, D], BF16, tag="dxb")
                last_step = step == steps - 1
                if with_ar:
                    arin = drp.tile([N, D], BF16, tag="arin")
                    arout = drp.tile([N, D], BF16, tag="arout")
                for nt in range(NT):
                    ms = stats.tile([P, 4], F32, tag="ms")
                    pt = ps.tile([P, 512], F32R, tag="ps")
                    for dt in range(4):
                        nc.tensor.transpose(pt[:, dt * P : (dt + 1) * P], dgTs[:, dt, nt * P : (nt + 1) * P], ident[:])
                    nc.vector.scalar_tensor_tensor(
                        out=dx[:, nt, 0:512], in0=f_(pt[:]), scalar=0.0, in1=xhat[:, nt, 0:512].bitcast(F32),
                        op0=OP.bypass, op1=OP.bypass, accum_out=ms[:, 0:1],
                    )
                    pt2 = ps.tile([P, 512], F32R, tag="ps")
                    for dt in range(4, DT):
                        nc.tensor.transpose(pt2[:, (dt - 4) * P : (dt - 3) * P], dgTs[:, dt, nt * P : (nt + 1) * P], ident[:])
                    nc.vector.scalar_tensor_tensor(
                        out=dx[:, nt, 512:768], in0=f_(pt2[:, :256]), scalar=0.0, in1=xhat[:, nt, 512:768].bitcast(F32),
                        op0=OP.bypass, op1=OP.bypass, accum_out=ms[:, 1:2],
                    )
                    # u2 = sum(dg*xhat), m1 = sum(dg)
                    prodA = scr.tile([P, D], F32, tag="prodA")
                    nc.vector.scalar_tensor_tensor(
                        out=prodA[:], in0=dx[:, nt, :], scalar=1.0, in1=f_(xhat[:, nt, :]),
                        op0=OP.mult, op1=OP.mult, accum_out=ms[:, 2:3],
                    )
                    nc.vector.tensor_tensor(out=ms[:, 3:4], in0=ms[:, 0:1], in1=ms[:, 1:2], op=OP.add)
                    ss = stats.tile([P, 2], F32, tag="ss")
                    nc.vector.tensor_scalar_mul(out=ss[:], in0=ms[:, 2:4], scalar1=1.0 / D)
                    # dxb = dg - xhat*<dg xhat> - <dg> (rstd folded into update)
                    nc.vector.ln_bwd_dx(
                        out=dxb[:, nt, :], dy=dx[:, nt, :], x_hat=f_(xhat[:, nt, :]),
                        mean_dyx=ss[:, 0:1], mean_dy=ss[:, 1:2], scale=1.0,
                    )
                    if with_ar:
                        nc.sync.dma_start(out=arin[nt * P : (nt + 1) * P, :], in_=dxb[:, nt, :])
                # ======== single pair AllReduce ========
                if with_ar:
                    nc.gpsimd.collective_compute(
                        "AllReduce", OP.add, replica_groups=REPLICA_GROUPS,
                        ins=[arin.opt()], outs=[arout.opt()],
                    )
                    for nt in range(NT):
                        nc.sync.dma_start(out=dxb[:, nt, :], in_=arout[nt * P : (nt + 1) * P, :])
                # ======== tail pass 2: update (+fused stats) + next LN-fwd ========
                if not last_step:
                    rstd_n = stats.tile([P, NT], F32, tag="rstd")
                import contextlib as _cl

                for nt in range(NT):
                    # chunk 0's whole chain runs at high priority so the next
                    # step's PE work unblocks as early as possible
                    prio = tc.high_priority() if nt == 0 else _cl.nullcontext()
                    with prio:
                        # x += (alpha*rstd) * dxb; accum gives sum(x_new) free
                        arr = stats.tile([P, 1], F32, tag="arr")
                        nc.vector.tensor_scalar_mul(out=arr[:], in0=rstd[:, nt : nt + 1], scalar1=ALPHA)
                        sums = stats.tile([P, 2], F32, tag="sums")
                        nc.vector.scalar_tensor_tensor(
                            out=x_sb[:, nt, :], in0=dxb[:, nt, :], scalar=arr[:], in1=x_sb[:, nt, :],
                            op0=OP.mult, op1=OP.add,
                            accum_out=sums[:, 0:1] if not last_step else None,
                        )
                        if not last_step:
                            # sum(x_new^2) on the (tail-idle) Scalar engine
                            xsq = scr.tile([P, D], BF16, tag="xsq")
                            nc.scalar.activation(
                                out=xsq[:], in_=x_sb[:, nt, :], func=AF.Square,
                                accum_out=sums[:, 1:2],
                            )
                            mrow = stats.tile([P, 2], F32, tag="mrow")
                            nc.vector.tensor_scalar_mul(out=mrow[:], in0=sums[:], scalar1=1.0 / D)
                            msq = stats.tile([P, 1], F32, tag="msq")
                            nc.vector.tensor_tensor(out=msq[:], in0=mrow[:, 0:1], in1=mrow[:, 0:1], op=OP.mult)
                            varc = stats.tile([P, 1], F32, tag="varc")
                            nc.vector.tensor_tensor(out=varc[:], in0=mrow[:, 1:2], in1=msq[:], op=OP.subtract)
                            ln_rstd(nt, varc[:], rstd_n)
                            ln_xhat(nt, mrow[:, 0:1], rstd_n, xhat)
                if not last_step:
                    rstd = rstd_n

            for nt in range(NT):
                nc.sync.dma_start(out=x_out[nt * P : (nt + 1) * P, :], in_=x_sb[:, nt, :])

    nc.compile()
    return nc


def _prep_inputs(x, gamma, delta, Wq, Wk, xi):
    """Build the 8 per-core input dicts (host-side sharding + weight folding)."""
    assert np.allclose(delta, 0.0), "kernel requires delta == 0"
    beta_sqrt = np.float32(1.0 / np.sqrt(np.sqrt(np.float32(HD))))
    g = gamma.astype(np.float32)
    import ml_dtypes

    bf = ml_dtypes.bfloat16
    in_maps = []
    for c in range(8):
        b, j = c // 2, c % 2
        hs = slice(j * HL, (j + 1) * HL)
        wq_l = (Wq[hs] * g[None, :, None]).transpose(1, 0, 2).reshape(D, EW)
        wk_l = (Wk[hs] * g[None, :, None]).transpose(1, 0, 2).reshape(D, EW)
        wqt_l = (Wq[hs] * g[None, :, None]).transpose(0, 2, 1).reshape(EW, D)
        wkt_l = (Wk[hs] * g[None, :, None]).transpose(0, 2, 1).reshape(EW, D)
        xi_l = xi[j * ML : (j + 1) * ML] * g[None, :]
        in_maps.append(
            {
                "x": np.ascontiguousarray(x[b]),
                "wq": np.ascontiguousarray(wq_l * beta_sqrt).astype(bf),
                "wk": np.ascontiguousarray(wk_l * beta_sqrt).astype(bf),
                "wqt": np.ascontiguousarray(wqt_l / beta_sqrt).astype(bf),
                "wkt": np.ascontiguousarray(wkt_l / beta_sqrt).astype(bf),
                "xi": np.ascontiguousarray(xi_l).astype(bf),
                "xit": np.ascontiguousarray(xi_l.T).astype(bf),
            }
        )
    return in_maps


_NC_CACHE = {}


def _get_nc(steps=STEPS, with_ar=True):
    key = (steps, with_ar)
    if key not in _NC_CACHE:
        _NC_CACHE[key] = build_kernel(steps, with_ar)
    return _NC_CACHE[key]


def kernel(x, gamma, delta, Wq, Wk, xi):
    from concourse.bass_utils import run_bass_kernel_spmd

    x = np.asarray(x, dtype=np.float32)
    in_maps = _prep_inputs(
        x,
        np.asarray(gamma, np.float32),
        np.asarray(delta, np.float32),
        np.asarray(Wq, np.float32),
        np.asarray(Wk, np.float32),
        np.asarray(xi, np.float32),
    )
    nc = _get_nc()
    res = run_bass_kernel_spmd(nc, in_maps, list(range(8)))
    out = np.stack([res.results[2 * b]["x_out"] for b in range(B)], axis=0)
    return out.astype(np.float32)
